# revision 14
# baseline (speedup 1.0000x reference)
"""Trainium2 Bass kernel for nn_MultiHeadAttention_48086453846410 (fp8).

Reference (heads folded into seq axis, softmax over FULL L = seq*heads keys):
    qp = (q @ wk_w.T + wk_b).reshape(bs, L, d)   # swapped wk/wq, faithful
    kp = (k @ wq_w.T + wq_b).reshape(bs, L, d)
    vp = (v @ wv_w.T + wv_b).reshape(bs, L, d)
    scores = qp @ kp.T / sqrt(d); attn = softmax(scores, -1)
    out = (attn @ vp).reshape(bs, seq, d*heads) @ out_w.T + out_b

Sharding: 8 cores = (batch b) x (seq half). Each core owns 256 query rows
(2048 l-rows), softmax over keys -> no collectives.

Speed strategy (cost model): fp8e4 DoubleRow matmuls (0.5 cyc/out-col).
Elementwise work spread over Act/DVE/Pool with Act reserved for the exp
stream in phase B (exp only runs on Act). Scores accumulate in 2-bank
[128,1024] psum tiles so exp/r-sub run as 1024-col ops (amortizes the
~370ns Act access-latency per instruction). po (attn@v accum) is single-
buffered; accumulation groups open via start=True on the first write per
bank (lazy zero-region semantics), no zero matmuls. Out-projection for
the first 128 output rows accumulates per-slice into a spare psum bank.

Precision strategy (gate 2e-2; baseline measured 0.0122 on HW):
 - weights scaled x64 on host before fp8 cast (w std 0.02 is subnormal).
 - attn weights: r = exp(s) - 1 (std 0.2) matmul'd in fp8; the "1 * vp"
   mean flow restored exactly via host-computed colsum through a bf16
   fold matmul: po' = po + colsum*1 - obar*Z.
 - out-projection mean-extraction: delta = (o - obar) in fp8; mean path
   obar @ ow.T + out_b = b_eff host-exact.  Z = 4096 + sum(r) from DR
   ones-matmuls (exact fp32 psum).
 - fp8 scales: weights x64, projections x4, r x8, delta x2048/Z.

Structure: phase A projections (4-bank rotating psum pool, epilogues on
DVE/Act); slice-0 scores emitted behind A2 heads, slice-1 m=0..3 behind
A3 heads. Phase B: 8 slices (one head's 256 l-cols each) of 8 merged
score tiles (512 keys each): 8 DR matmuls -> exp [128,1024] (Act) ->
r-sub (Pool/DVE) -> deferred attnv+Z DR matmuls. Slice boundary: -Z fold
(DVE) -> csob fold matmuls -> 1/Z (DVE) -> dT8 delta extract (DVE) ->
early out-proj (st=0). Tail: st=1 out-proj into the freed psz bank.
"""

import math
import sys

for _p in ("/opt/trn_rl_repo",):
    if _p not in sys.path:
        sys.path.insert(0, _p)

import numpy as np
import ml_dtypes

BS, SEQ, D, HEADS = 4, 512, 512, 8
NCORES = 8
S = SEQ // 2            # 256 query seq rows per core
JT = HEADS * D // 128   # 32 j-tiles of the 4096 projection dim
DT = D // 128           # 4 d-tiles of the 512 contraction dim
WS = 64.0               # host fp8 weight scale
NP_BF16 = ml_dtypes.bfloat16
NP_F8 = ml_dtypes.float8_e4m3

_CACHE = {}


def _build_program():
    from concourse import bacc
    import concourse.mybir as mybir
    import concourse.tile as tile
    from concourse.dt import dt

    f32 = dt.float32
    b16 = dt.bfloat16
    f8 = dt.float8e4
    Act = mybir.ActivationFunctionType
    Alu = mybir.AluOpType
    DR = mybir.MatmulPerfMode.DoubleRow

    nc = bacc.Bacc(None, target_bir_lowering=False, debug=False,
                   num_devices=NCORES)

    def din(name, shape, dty):
        return nc.dram_tensor(name, shape, dty, kind="ExternalInput").ap()

    q8T = din("q8T", [D, S], f8)                # q[b,half].T  (d, s)
    k8T = din("k8T", [D, SEQ], f8)              # k[b].T       (d, t)
    v8T = din("v8T", [D, SEQ], f8)              # v[b].T       (d, t)
    wk8T = din("wk8T", [D, HEADS * D], f8)      # 64*wk_w.T    (d, j)
    wq8T = din("wq8T", [D, HEADS * D], f8)
    wv8T = din("wv8T", [D, HEADS * D], f8)
    ow8T = din("ow8T", [HEADS * D, D], f8)      # 64*out_w.T   (c, r)
    wkb = din("wkb", [128, JT], f32)            # wk_b.reshape(JT,128).T
    wqb = din("wqb", [128, JT], f32)
    wvb8 = din("wvb8", [128, 2 * HEADS * D], f8)  # [64*wv_b repl | zeros]
    ones8d = din("ones8", [128, 256], f8)       # DR ones (Z matmul lhsT)
    onescol8d = din("onescol8", [128, 256], f8)  # [1/16 | 0] bias-fold lhsT
    csobd = din("csob", [64, SEQ], b16)    # r0=obar r32=colsum rest 0
    zfoldd = din("zfoldi", [64, SEQ], b16)      # r32=ones rest 0
    b_effd = din("b_eff", [128, D], f32)        # obar@owT+out_b replicated
    out = nc.dram_tensor("out", [S, D], f32, kind="ExternalOutput").ap()

    inv_sqrt_d = 1.0 / math.sqrt(D)

    with tile.TileContext(nc) as tc:
        with (
            tc.tile_pool(name="big", bufs=1) as bp,
            tc.tile_pool(name="exp", bufs=6) as ep,
            tc.tile_pool(name="r8p", bufs=14) as rp,
            tc.tile_pool(name="zrp", bufs=2) as zp,
            tc.tile_pool(name="psc", bufs=2, space="PSUM") as psc,
        ):
            # ---------------- DMAs (first-needed-first) ----------------
            def dma(dst, src):
                nc.sync.dma_start(out=dst, in_=src)

            q8T_sb = bp.tile([128, DT * S], f8, tag="q8T")
            dma(q8T_sb.rearrange("p (t n) -> p t n", n=S),
                q8T.rearrange("(t p) n -> p t n", p=128))
            wkd = wk8T.rearrange("(t p) n -> p t n", p=128)
            wk_q = []
            for c in range(4):
                t = bp.tile([128, DT * 1024], f8, tag=f"wk{c}",
                            name=f"wk{c}")
                tv = t.rearrange("p (t n) -> p t n", n=1024)
                for hc in range(2):
                    dma(tv[:, :, hc * 512:(hc + 1) * 512],
                        wkd[:, :, c * 1024 + hc * 512:
                             c * 1024 + (hc + 1) * 512])
                wk_q.append(t)
                if c == 0:
                    wkb_sb = bp.tile([128, JT], f32, tag="wkb")
                    dma(wkb_sb, wkb)
            k8T_sb = bp.tile([128, DT * SEQ], f8, tag="k8T")
            dma(k8T_sb.rearrange("p (t n) -> p t n", n=SEQ),
                k8T.rearrange("(t p) n -> p t n", p=128))
            wqb_sb = bp.tile([128, JT], f32, tag="wqb")
            dma(wqb_sb, wqb)
            wqd = wq8T.rearrange("(t p) n -> p t n", p=128)
            wq_q = []
            for c in range(4):
                t = bp.tile([128, DT * 1024], f8, tag=f"wq{c}",
                            name=f"wq{c}")
                dma(t.rearrange("p (t n) -> p t n", n=1024),
                    wqd[:, :, c * 1024:(c + 1) * 1024])
                wq_q.append(t)
            v8T_sb = bp.tile([128, DT * SEQ], f8, tag="v8T")
            dma(v8T_sb.rearrange("p (t n) -> p t n", n=SEQ),
                v8T.rearrange("(t p) n -> p t n", p=128))
            wvb8_sb = bp.tile([128, 2 * HEADS * D], f8, tag="wvb8")
            dma(wvb8_sb, wvb8)
            onescol8 = bp.tile([128, 256], f8, tag="onescol8")
            dma(onescol8, onescol8d)
            ones8 = bp.tile([128, 256], f8, tag="ones8")
            dma(ones8, ones8d)
            wvd = wv8T.rearrange("(t p) n -> p t n", p=128)
            wv_q = []
            for c in range(4):
                t = bp.tile([128, DT * 1024], f8, tag=f"wv{c}",
                            name=f"wv{c}")
                dma(t.rearrange("p (t n) -> p t n", n=1024),
                    wvd[:, :, c * 1024:(c + 1) * 1024])
                wv_q.append(t)
            csob = bp.tile([64, SEQ], b16, tag="csob")
            dma(csob, csobd)
            zfold = bp.tile([64, SEQ], b16, tag="zfold")
            dma(zfold, zfoldd)
            ow_sb = bp.tile([128, JT * D], f8, tag="ow")
            owv = ow_sb.rearrange("p (t n) -> p t n", n=D)
            owd = ow8T.rearrange("(t p) n -> p t n", p=128)
            for c in range(4):
                dma(owv[:, 8 * c:8 * (c + 1), :],
                    owd[:, 8 * c:8 * (c + 1), :])
            b_eff = bp.tile([128, D], f32, tag="beff")
            dma(b_eff, b_effd)

            # ---------------- persistent SBUF state ----------------
            qp8 = [bp.tile([128, DT * S], f8, tag=f"qp{h}", name=f"qp{h}")
                   for h in range(HEADS)]          # cols dtj*S + s
            kp8 = [bp.tile([128, DT * SEQ], f8, tag=f"kp{g}", name=f"kp{g}")
                   for g in range(HEADS)]          # cols dt*SEQ + t
            vp8 = [bp.tile([128, DT * 512], f8, tag=f"vg{g}", name=f"vg{g}")
                   for g in range(HEADS)]          # cols tt*512 + e
            dT8 = bp.tile([128, DT * 2048], f8, tag="dT8")      # et*2048+l
            out_sb = bp.tile([128, 2 * D], f32, tag="outsb")
            tmp_sb = bp.tile([128, 2 * D], f32, tag="tmpsb")

            # phase-A epilogues: psum readers -> DVE or Act only. A1 runs
            # before the exp stream starts (Act idle) -> 50/50; during
            # A2/A3 Act carries the early exps -> 1/3 Act.
            epi_state = {"cycle": ["a", "v"], "i": 0}

            def epilogue(dst, ps, bias_ap):
                cyc = epi_state["cycle"]
                e = cyc[epi_state["i"] % len(cyc)]
                epi_state["i"] += 1
                if e == "a":
                    nc.scalar.activation(dst, ps, Act.Identity,
                                         bias=(bias_ap if bias_ap is not None
                                               else 0.0),
                                         scale=4.0 / WS)
                else:
                    if bias_ap is not None:
                        nc.vector.tensor_scalar(dst, ps, 4.0 / WS, bias_ap,
                                                Alu.mult, Alu.add)
                    else:
                        nc.vector.tensor_scalar(dst, ps, 4.0 / WS, None,
                                                Alu.mult)

            q8vv = q8T_sb.rearrange("p (a x) -> p a x", x=S)
            v8vv = v8T_sb.rearrange("p (a x) -> p a x", x=SEQ)
            k8vv = k8T_sb.rearrange("p (a x) -> p a x", x=SEQ)
            wvb8v = wvb8_sb.rearrange("p (a x) -> p a x", x=HEADS * D)
            oc8v = onescol8.rearrange("p (a x) -> p a x", x=128)
            on8v = ones8.rearrange("p (a x) -> p a x", x=128)
            dTv = dT8.rearrange("p (a x) -> p a x", x=2048)
            owvv = ow_sb.rearrange("p (a x) -> p a x", x=D)
            qpv8 = [qp8[h].rearrange("p (a x) -> p a x", x=S)
                    for h in range(HEADS)]

            # merged scores tile: head m's full 512 keys x the slice's 256
            # l-cols, in a 2-bank [128,1024] psum tile (blocks = 4 key
            # tiles). exp+rsub run as single 1024-col ops.
            rsub_i = [0]

            def emit_merged(ls, m, pend, attnv):
                kpv = kp8[m].rearrange("p (a x) -> p a x", x=SEQ)
                qpv = qpv8[ls]
                psp = psc.tile([128, 1024], f32, tag="sc",
                               name=f"sc_{ls}_{m}")
                for tt in range(4):
                    for dtp in range(2):
                        nc.tensor.matmul(
                            psp[:, tt * 256:(tt + 1) * 256],
                            lhsT=kpv[:, 2 * dtp:2 * dtp + 2,
                                     tt * 128:(tt + 1) * 128],
                            rhs=qpv[:, 2 * dtp:2 * dtp + 2, :],
                            start=(dtp == 0), stop=(dtp == 1),
                            perf_mode=DR)
                ex = ep.tile([128, 1024], b16, tag="ex",
                             name=f"ex_{ls}_{m}")
                nc.scalar.activation(ex, psp, Act.Exp, bias=0.0,
                                     scale=inv_sqrt_d / 16.0)
                r8 = rp.tile([128, 1024], f8, tag="r8",
                             name=f"r8_{ls}_{m}")
                # r-sub: SBUF-only -> Pool takes most, DVE 3-of-8 (late m
                # only, so boundary dT8 extracts aren't queued behind them)
                rsub_i[0] += 1
                reng = nc.vector if (ls >= 2 and m in (4, 5, 6, 7)) \
                    else nc.gpsimd
                reng.tensor_scalar(r8, ex, -1.0, 8.0, Alu.add, Alu.mult)
                r8v = r8.rearrange("p (a x) -> p a x", x=256)
                # pend holds half-tiles (kt2 granularity) so drained attnv
                # bursts between scores stay short (~270ns) and never
                # head-of-line-block the scores feeding the Act exp stream
                pend.append((m, 0, r8v))
                pend.append((m, 1, r8v))
                if attnv is not None:
                    # taper: drain harder near slice end so the boundary
                    # flush (which gates the psz z-chain) stays tiny
                    if ls == 7 and m >= 5:
                        skew = 0
                    else:
                        skew = {6: 4, 7: 2}.get(m, 6)
                    while len(pend) > skew:
                        attnv(*pend.pop(0))

            # ---------------- phase A ----------------
            pend0, pend1 = [], []
            with tc.tile_pool(name="pa2", bufs=4, space="PSUM") as pa2:
                epi_state["cycle"] = ["a", "v"]
                for h in range(HEADS):          # A1: qpT
                    for dtj in range(DT):
                        jt = h * DT + dtj
                        wkq = wk_q[jt // 8].rearrange("p (a x) -> p a x",
                                                      x=1024)
                        jo = (jt % 8) * 128
                        ps = pa2.tile([128, 512], f32, tag="asc",
                                      name=f"a1_{jt}")
                        for dtp in range(2):
                            nc.tensor.matmul(
                                ps[:, 0:S],
                                lhsT=wkq[:, 2 * dtp:2 * dtp + 2, jo:jo + 128],
                                rhs=q8vv[:, 2 * dtp:2 * dtp + 2, :],
                                start=(dtp == 0), stop=(dtp == 1),
                                perf_mode=DR)
                        epilogue(qp8[h][:, dtj * S:(dtj + 1) * S],
                                 ps[:, 0:S], wkb_sb[:, jt:jt + 1])
                epi_state["cycle"] = ["v", "a", "v"]
                for g in range(HEADS):          # A2 kp per head
                    for dtj in range(DT):
                        jt = g * DT + dtj
                        wqq = wq_q[jt // 8].rearrange("p (a x) -> p a x",
                                                      x=1024)
                        jo = (jt % 8) * 128
                        ps = pa2.tile([128, 512], f32, tag="asc",
                                      name=f"a2_{jt}")
                        for dtp in range(2):
                            nc.tensor.matmul(
                                ps,
                                lhsT=wqq[:, 2 * dtp:2 * dtp + 2, jo:jo + 128],
                                rhs=k8vv[:, 2 * dtp:2 * dtp + 2, :],
                                start=(dtp == 0), stop=(dtp == 1),
                                perf_mode=DR)
                        epilogue(kp8[g][:, dtj * SEQ:(dtj + 1) * SEQ], ps,
                                 wqb_sb[:, jt:jt + 1])
                    emit_merged(0, g, pend0, None)
                for g in range(HEADS):          # A3 vp per head
                    wvq = wv_q[g // 2].rearrange("p (a x) -> p a x", x=1024)
                    go = (g % 2) * 512
                    for tt in range(DT):
                        ps = pa2.tile([128, 512], f32, tag="asc",
                                      name=f"a3_{g}_{tt}")
                        for dtp in range(2):
                            nc.tensor.matmul(
                                ps,
                                lhsT=v8vv[:, 2 * dtp:2 * dtp + 2,
                                          tt * 128:(tt + 1) * 128],
                                rhs=wvq[:, 2 * dtp:2 * dtp + 2, go:go + 512],
                                start=(dtp == 0), stop=False, perf_mode=DR)
                        nc.tensor.matmul(            # fold 64*wv_b
                            ps, lhsT=oc8v[:, 0:2, :],
                            rhs=wvb8v[:, 0:2, g * 512:(g + 1) * 512],
                            start=False, stop=True, perf_mode=DR)
                        epilogue(vp8[g][:, tt * 512:(tt + 1) * 512], ps, None)
                    if g % 2 == 1:              # slice-1 scores m=0..3
                        emit_merged(1, g // 2, pend1, None)

            # ---------------- phase B ----------------
            with tc.tile_pool(name="psb", bufs=1, space="PSUM") as psb:
                po = psb.tile([128, 1024], f32, tag="po")
                pszt = psb.tile([128, 512], f32, tag="psz")
                psz = pszt[:, 0:256]
                psc0 = psb.tile([128, 512], f32, tag="pso")

                def make_attnv(ls):
                    def attnv(m, kt2, r8v):
                        vpg = vp8[m].rearrange("p (a x) -> p a x", x=512)
                        first = (m == 0 and kt2 == 0)
                        for et in range(DT):
                            nc.tensor.matmul(
                                po[:, et * 256:(et + 1) * 256],
                                lhsT=vpg[:, 2 * kt2:2 * kt2 + 2,
                                         et * 128:(et + 1) * 128],
                                rhs=r8v[:, 2 * kt2:2 * kt2 + 2, :],
                                start=(first and et in (0, 2)),
                                stop=False, perf_mode=DR,
                                skip_group_check=True)
                        nc.tensor.matmul(
                            psz, lhsT=on8v[:, 0:2, :],
                            rhs=r8v[:, 2 * kt2:2 * kt2 + 2, :],
                            start=first,
                            stop=(m == 7 and kt2 == 1),
                            perf_mode=DR, skip_group_check=True)
                    return attnv

                def zchain(ls):
                    # -Z fold row + 1/Z, all DVE (psz readers)
                    nc.vector.tensor_scalar(
                        zfold[0:32, 0:256], psz[0:32, :], -1.0 / 8.0,
                        -4096.0, Alu.mult, Alu.add)
                    t1 = zp.tile([128, 256], f32, tag="t1", name=f"t1_{ls}")
                    nc.vector.tensor_scalar(t1, psz, 1.0 / 512.0, 64.0,
                                            Alu.mult, Alu.add)
                    zr = zp.tile([128, 256], f32, tag="zr", name=f"zr_{ls}")
                    nc.vector.reciprocal(zr, t1)
                    return zr

                def folds(ls):
                    for et in range(DT):
                        nc.tensor.matmul(
                            po[:, et * 256:(et + 1) * 256],
                            lhsT=csob[:, et * 128:(et + 1) * 128],
                            rhs=zfold[:, 0:256], start=False, stop=True,
                            skip_group_check=True)

                def extract(ls, zr):
                    for et in range(DT):
                        nc.vector.tensor_tensor(
                            out=dT8[:, et * 2048 + ls * 256:
                                    et * 2048 + (ls + 1) * 256],
                            in0=po[:, et * 256:(et + 1) * 256], in1=zr,
                            op=Alu.mult)

                def outproj0(ls):
                    # early out-projection, output rows 0:128 (st=0)
                    for etp in range(2):
                        nc.tensor.matmul(
                            psc0,
                            lhsT=dTv[:, 2 * etp:2 * etp + 2,
                                     ls * 256:ls * 256 + 128],
                            rhs=owvv[:, ls * DT + 2 * etp:
                                     ls * DT + 2 * etp + 2, :],
                            start=(ls == 0 and etp == 0),
                            stop=(ls == 7 and etp == 1),
                            perf_mode=DR, skip_group_check=True)

                def transition(ls, pend, attnv, nexts):
                    # slice-ls boundary with the next slice's first score
                    # tiles interleaved so neither PE nor Act is ever
                    # queued head-of-line behind the boundary chain.
                    # outproj0(ls-1) runs here: dT8(ls-1) is long done, so
                    # it can never stall PE on the extract chain.
                    nx = list(nexts)
                    if nx:
                        nx.pop(0)()         # feed Act before the flush
                    cnt = 0
                    while len(pend) > 2:
                        attnv(*pend.pop(0))
                        cnt += 1
                        if cnt % 4 == 0 and nx:
                            nx.pop(0)()
                    if nx:
                        nx.pop(0)()
                    for args in pend:       # last halves close psz
                        attnv(*args)
                    pend.clear()
                    if ls > 0:
                        outproj0(ls - 1)
                    zr = zchain(ls)
                    if nx:
                        nx.pop(0)()
                    folds(ls)
                    if nx:
                        nx.pop(0)()
                    extract(ls, zr)
                    while nx:
                        nx.pop(0)()
                    if ls == 7:
                        outproj0(7)

                def em(ls, m, pend, attnv=None):
                    return lambda: emit_merged(ls, m, pend, attnv)

                atts = [make_attnv(ls) for ls in range(2 * 4)]
                pend2 = []
                emit_merged(1, 4, pend1, None)
                transition(0, pend0, atts[0],
                           [em(1, 5, pend1), em(1, 6, pend1),
                            em(1, 7, pend1)])
                transition(1, pend1, atts[1],
                           [em(2, 0, pend2), em(2, 1, pend2),
                            em(2, 2, pend2), em(2, 3, pend2)])
                pend = pend2
                for ls in range(2, 8):
                    for m in range(4, 8):
                        emit_merged(ls, m, pend, atts[ls])
                    if ls < 7:
                        pend_next = []
                        transition(ls, pend, atts[ls],
                                   [em(ls + 1, 0, pend_next),
                                    em(ls + 1, 1, pend_next),
                                    em(ls + 1, 2, pend_next),
                                    em(ls + 1, 3, pend_next)])
                        pend = pend_next
                    else:
                        transition(ls, pend, atts[ls], [])

                # tail: out rows 128:256 (st=1) into the freed psz bank
                for hh in range(8):
                    for etp in range(2):
                        nc.tensor.matmul(
                            pszt,
                            lhsT=dTv[:, 2 * etp:2 * etp + 2,
                                     hh * 256 + 128:hh * 256 + 256],
                            rhs=owvv[:, hh * DT + 2 * etp:
                                     hh * DT + 2 * etp + 2, :],
                            start=(hh == 0 and etp == 0),
                            stop=(hh == 7 and etp == 1),
                            perf_mode=DR, skip_group_check=True)
                nc.scalar.activation(
                    tmp_sb[:, 0:D], psc0,
                    Act.Identity, bias=0.0, scale=1.0 / (2048.0 * WS))
                nc.vector.tensor_tensor(
                    out=out_sb[:, 0:D], in0=tmp_sb[:, 0:D], in1=b_eff,
                    op=Alu.add)
                nc.sync.dma_start(out=out[0:128, :], in_=out_sb[:, 0:D])
                nc.scalar.activation(
                    tmp_sb[:, D:2 * D], pszt,
                    Act.Identity, bias=0.0, scale=1.0 / (2048.0 * WS))
                nc.vector.tensor_tensor(
                    out=out_sb[:, D:2 * D], in0=tmp_sb[:, D:2 * D],
                    in1=b_eff, op=Alu.add)
                nc.sync.dma_start(out=out[128:256, :],
                                  in_=out_sb[:, D:2 * D])

    nc.compile()
    return nc


def _get_program():
    if "nc" not in _CACHE:
        _CACHE["nc"] = _build_program()
    return _CACHE["nc"]


def _prep_shared(inputs):
    f8 = NP_F8
    c = np.ascontiguousarray
    f32 = np.float32

    def t8(x, scale=1.0):
        return c((np.asarray(x, f32) * scale).T).astype(f8)

    # bias fold: sum_k (1/16) * (8*wv_b) over 128 partitions = 64*wv_b.
    # 1/16 stays fp8-normal (1/128 would be subnormal -> FTZ risk).
    wvb = np.zeros((128, 2 * HEADS * D), f32)
    wvb[:, :HEADS * D] = np.asarray(inputs["wv_b"], f32)[None, :] * 8.0
    onescol = np.zeros((128, 256), f32)
    onescol[:, :128] = 1.0 / 16.0
    zfold = np.zeros((64, SEQ), f32)
    zfold[32, :] = 1.0
    return {
        "wk8T": t8(inputs["wk_w"], WS),
        "wq8T": t8(inputs["wq_w"], WS),
        "wv8T": t8(inputs["wv_w"], WS),
        "ow8T": t8(inputs["out_w"], WS),
        # biases x4: projections are stored 4x-scaled in fp8 (subnormal
        # avoidance); epilogue computes ps*4/WS + 4*b.
        "wkb": c(np.asarray(inputs["wk_b"], f32).reshape(JT, 128).T) * 4.0,
        "wqb": c(np.asarray(inputs["wq_b"], f32).reshape(JT, 128).T) * 4.0,
        "wvb8": wvb.astype(f8),
        "ones8": np.ones((128, 256), f8),
        "onescol8": onescol.astype(f8),
        "zfoldi": zfold.astype(NP_BF16),
    }


def _make_in_maps(inputs):
    f8 = NP_F8
    c = np.ascontiguousarray
    shared = _prep_shared(inputs)
    q = np.asarray(inputs["q"], np.float32)
    k = np.asarray(inputs["k"], np.float32)
    v = np.asarray(inputs["v"], np.float32)
    wv_w = np.asarray(inputs["wv_w"], np.float64)
    wv_b = np.asarray(inputs["wv_b"], np.float64)
    ow = np.asarray(inputs["out_w"], np.float64)
    ob = np.asarray(inputs["out_b"], np.float64)

    per_batch = []
    for b in range(BS):
        vsum = v[b].astype(np.float64).sum(axis=0)
        colsum = (vsum @ wv_w.T + SEQ * wv_b).reshape(HEADS, D).sum(axis=0)
        cs_bf = colsum.astype(NP_BF16)
        obar_bf = (colsum / (SEQ * HEADS)).astype(NP_BF16)
        # x32: attn psum is at scale 8(r) * 4(vp) = 32
        csob = np.zeros((64, SEQ), np.float32)
        csob[0, :] = obar_bf.astype(np.float32) * 32.0
        csob[32, :] = cs_bf.astype(np.float32) * 32.0
        b_eff = (np.tile(obar_bf.astype(np.float64), HEADS) @ ow.T + ob
                 ).astype(np.float32)
        per_batch.append({
            "k8T": c(k[b].T).astype(f8),
            "v8T": c(v[b].T).astype(f8),
            "csob": csob.astype(NP_BF16),
            "b_eff": np.broadcast_to(b_eff[None, :], (128, D)).copy(),
        })

    in_maps = []
    for core in range(NCORES):
        b, half = divmod(core, 2)
        m = dict(shared)
        m.update(per_batch[b])
        m["q8T"] = c(q[b, half * S:(half + 1) * S, :].T).astype(f8)
        in_maps.append(m)
    return in_maps


def kernel(**inputs):
    from concourse.bass_utils import run_bass_kernel_spmd

    nc = _get_program()
    in_maps = _make_in_maps(inputs)
    res = run_bass_kernel_spmd(nc, in_maps, core_ids=list(range(NCORES)))
    _CACHE["last_results"] = res
    out = np.empty((BS, SEQ, D), np.float32)
    for core in range(NCORES):
        b, half = divmod(core, 2)
        out[b, half * S:(half + 1) * S, :] = res.results[core]["out"]
    return out


if __name__ == "__main__":
    rng = np.random.default_rng(0)
    fake = {
        "q": rng.standard_normal((BS, SEQ, D)).astype(np.float32),
        "k": rng.standard_normal((BS, SEQ, D)).astype(np.float32),
        "v": rng.standard_normal((BS, SEQ, D)).astype(np.float32),
        "wq_w": (rng.standard_normal((D * HEADS, D)) * 0.02).astype(np.float32),
        "wq_b": (rng.standard_normal((D * HEADS,)) * 0.02).astype(np.float32),
        "wk_w": (rng.standard_normal((D * HEADS, D)) * 0.02).astype(np.float32),
        "wk_b": (rng.standard_normal((D * HEADS,)) * 0.02).astype(np.float32),
        "wv_w": (rng.standard_normal((D * HEADS, D)) * 0.02).astype(np.float32),
        "wv_b": (rng.standard_normal((D * HEADS,)) * 0.02).astype(np.float32),
        "out_w": (rng.standard_normal((D, D * HEADS)) * 0.02).astype(np.float32),
        "out_b": (rng.standard_normal((D,)) * 0.02).astype(np.float32),
    }
    o = kernel(**fake)
    print("kernel ran, out shape", o.shape, "std", o.std())


# revision 18
# speedup vs baseline: 1.0357x; 1.0357x over previous
"""Trainium2 Bass kernel for nn_MultiHeadAttention_48086453846410 (fp8).

Reference (heads folded into seq axis, softmax over FULL L = seq*heads keys):
    qp = (q @ wk_w.T + wk_b).reshape(bs, L, d)   # swapped wk/wq, faithful
    kp = (k @ wq_w.T + wq_b).reshape(bs, L, d)
    vp = (v @ wv_w.T + wv_b).reshape(bs, L, d)
    scores = qp @ kp.T / sqrt(d); attn = softmax(scores, -1)
    out = (attn @ vp).reshape(bs, seq, d*heads) @ out_w.T + out_b

Sharding: 8 cores = (batch b) x (seq half). Each core owns 256 query rows
(2048 l-rows), softmax over keys -> no collectives.

Speed strategy (cost model): fp8e4 DoubleRow matmuls (0.5 cyc/out-col).
Elementwise work spread over Act/DVE/Pool with Act reserved for the exp
stream in phase B (exp only runs on Act). Scores accumulate in 2-bank
[128,1024] psum tiles so exp/r-sub run as 1024-col ops (amortizes the
~370ns Act access-latency per instruction). po (attn@v accum) is single-
buffered; accumulation groups open via start=True on the first write per
bank (lazy zero-region semantics), no zero matmuls. Out-projection for
the first 128 output rows accumulates per-slice into a spare psum bank.

Precision strategy (gate 2e-2; baseline measured 0.0122 on HW):
 - weights scaled x64 on host before fp8 cast (w std 0.02 is subnormal).
 - attn weights: r = exp(s) - 1 (std 0.2) matmul'd in fp8; the "1 * vp"
   mean flow restored exactly via host-computed colsum through a bf16
   fold matmul: po' = po + colsum*1 - obar*Z.
 - out-projection mean-extraction: delta = (o - obar) in fp8; mean path
   obar @ ow.T + out_b = b_eff host-exact.  Z = 4096 + sum(r) from DR
   ones-matmuls (exact fp32 psum).
 - fp8 scales: weights x64, projections x4, r x8, delta x2048/Z.

Structure: phase A projections (4-bank rotating psum pool, epilogues on
DVE/Act); slice-0 scores emitted behind A2 heads, slice-1 m=0..3 behind
A3 heads. Phase B: 8 slices (one head's 256 l-cols each) of 8 merged
score tiles (512 keys each): 8 DR matmuls -> exp [128,1024] (Act) ->
r-sub (Pool/DVE) -> deferred attnv+Z DR matmuls. Slice boundary: -Z fold
(DVE) -> csob fold matmuls -> 1/Z (DVE) -> dT8 delta extract (DVE) ->
early out-proj (st=0). Tail: st=1 out-proj into the freed psz bank.
"""

import math
import sys

for _p in ("/opt/trn_rl_repo",):
    if _p not in sys.path:
        sys.path.insert(0, _p)

import numpy as np
import ml_dtypes

BS, SEQ, D, HEADS = 4, 512, 512, 8
NCORES = 8
S = SEQ // 2            # 256 query seq rows per core
JT = HEADS * D // 128   # 32 j-tiles of the 4096 projection dim
DT = D // 128           # 4 d-tiles of the 512 contraction dim
WS = 64.0               # host fp8 weight scale
NP_BF16 = ml_dtypes.bfloat16
NP_F8 = ml_dtypes.float8_e4m3

_CACHE = {}


def _build_program():
    from concourse import bacc
    import concourse.mybir as mybir
    import concourse.tile as tile
    from concourse.dt import dt

    f32 = dt.float32
    b16 = dt.bfloat16
    f8 = dt.float8e4
    Act = mybir.ActivationFunctionType
    Alu = mybir.AluOpType
    DR = mybir.MatmulPerfMode.DoubleRow

    nc = bacc.Bacc(None, target_bir_lowering=False, debug=False,
                   num_devices=NCORES)

    def din(name, shape, dty):
        return nc.dram_tensor(name, shape, dty, kind="ExternalInput").ap()

    q8T = din("q8T", [D, S], f8)                # q[b,half].T  (d, s)
    k8T = din("k8T", [D, SEQ], f8)              # k[b].T       (d, t)
    v8T = din("v8T", [D, SEQ], f8)              # v[b].T       (d, t)
    wk8T = din("wk8T", [D, HEADS * D], f8)      # 64*wk_w.T    (d, j)
    wq8T = din("wq8T", [D, HEADS * D], f8)
    wv8T = din("wv8T", [D, HEADS * D], f8)
    ow8T = din("ow8T", [HEADS * D, D], f8)      # 64*out_w.T   (c, r)
    wkb = din("wkb", [128, JT], f32)            # wk_b.reshape(JT,128).T
    wqb = din("wqb", [128, JT], f32)
    wvb8 = din("wvb8", [128, 2 * HEADS * D], f8)  # [64*wv_b repl | zeros]
    ones8d = din("ones8", [128, 256], f8)       # DR ones (Z matmul lhsT)
    onescol8d = din("onescol8", [128, 256], f8)  # [1/16 | 0] bias-fold lhsT
    csobd = din("csob", [64, SEQ], b16)    # r0=obar r32=colsum rest 0
    zfoldd = din("zfoldi", [64, SEQ], b16)      # r32=ones rest 0
    b_effd = din("b_eff", [128, D], f32)        # obar@owT+out_b replicated
    out = nc.dram_tensor("out", [S, D], f32, kind="ExternalOutput").ap()

    inv_sqrt_d = 1.0 / math.sqrt(D)

    with tile.TileContext(nc) as tc:
        with (
            tc.tile_pool(name="big", bufs=1) as bp,
            tc.tile_pool(name="exp", bufs=6) as ep,
            tc.tile_pool(name="r8p", bufs=14) as rp,
            tc.tile_pool(name="zrp", bufs=2) as zp,
            tc.tile_pool(name="psc", bufs=2, space="PSUM") as psc,
        ):
            # ---------------- DMAs (first-needed-first) ----------------
            def dma(dst, src):
                nc.sync.dma_start(out=dst, in_=src)

            q8T_sb = bp.tile([128, DT * S], f8, tag="q8T")
            dma(q8T_sb.rearrange("p (t n) -> p t n", n=S),
                q8T.rearrange("(t p) n -> p t n", p=128))
            wkd = wk8T.rearrange("(t p) n -> p t n", p=128)
            wk_q = []
            for c in range(4):
                t = bp.tile([128, DT * 1024], f8, tag=f"wk{c}",
                            name=f"wk{c}")
                tv = t.rearrange("p (t n) -> p t n", n=1024)
                for hc in range(2):
                    dma(tv[:, :, hc * 512:(hc + 1) * 512],
                        wkd[:, :, c * 1024 + hc * 512:
                             c * 1024 + (hc + 1) * 512])
                wk_q.append(t)
                if c == 0:
                    wkb_sb = bp.tile([128, JT], f32, tag="wkb")
                    dma(wkb_sb, wkb)
            k8T_sb = bp.tile([128, DT * SEQ], f8, tag="k8T")
            dma(k8T_sb.rearrange("p (t n) -> p t n", n=SEQ),
                k8T.rearrange("(t p) n -> p t n", p=128))
            wqb_sb = bp.tile([128, JT], f32, tag="wqb")
            dma(wqb_sb, wqb)
            wqd = wq8T.rearrange("(t p) n -> p t n", p=128)
            wq_q = []
            for c in range(4):
                t = bp.tile([128, DT * 1024], f8, tag=f"wq{c}",
                            name=f"wq{c}")
                dma(t.rearrange("p (t n) -> p t n", n=1024),
                    wqd[:, :, c * 1024:(c + 1) * 1024])
                wq_q.append(t)
            v8T_sb = bp.tile([128, DT * SEQ], f8, tag="v8T")
            dma(v8T_sb.rearrange("p (t n) -> p t n", n=SEQ),
                v8T.rearrange("(t p) n -> p t n", p=128))
            wvb8_sb = bp.tile([128, 2 * HEADS * D], f8, tag="wvb8")
            dma(wvb8_sb, wvb8)
            onescol8 = bp.tile([128, 256], f8, tag="onescol8")
            dma(onescol8, onescol8d)
            ones8 = bp.tile([128, 256], f8, tag="ones8")
            dma(ones8, ones8d)
            wvd = wv8T.rearrange("(t p) n -> p t n", p=128)
            wv_q = []
            for c in range(4):
                t = bp.tile([128, DT * 1024], f8, tag=f"wv{c}",
                            name=f"wv{c}")
                dma(t.rearrange("p (t n) -> p t n", n=1024),
                    wvd[:, :, c * 1024:(c + 1) * 1024])
                wv_q.append(t)
            csob = bp.tile([64, SEQ], b16, tag="csob")
            dma(csob, csobd)
            zfold = bp.tile([64, SEQ], b16, tag="zfold")
            dma(zfold, zfoldd)
            ow_sb = bp.tile([128, JT * D], f8, tag="ow")
            owv = ow_sb.rearrange("p (t n) -> p t n", n=D)
            owd = ow8T.rearrange("(t p) n -> p t n", p=128)
            for c in range(4):
                dma(owv[:, 8 * c:8 * (c + 1), :],
                    owd[:, 8 * c:8 * (c + 1), :])
            b_eff = bp.tile([128, D], f32, tag="beff")
            dma(b_eff, b_effd)

            # ---------------- persistent SBUF state ----------------
            qp8 = [bp.tile([128, DT * S], f8, tag=f"qp{h}", name=f"qp{h}")
                   for h in range(HEADS)]          # cols dtj*S + s
            kp8 = [bp.tile([128, DT * SEQ], f8, tag=f"kp{g}", name=f"kp{g}")
                   for g in range(HEADS)]          # cols dt*SEQ + t
            vp8 = [bp.tile([128, DT * 512], f8, tag=f"vg{g}", name=f"vg{g}")
                   for g in range(HEADS)]          # cols tt*512 + e
            dT8 = bp.tile([128, DT * 2048], f8, tag="dT8")      # et*2048+l
            out_sb = bp.tile([128, 2 * D], f32, tag="outsb")
            tmp_sb = bp.tile([128, 2 * D], f32, tag="tmpsb")

            # phase-A epilogues: psum readers -> DVE or Act only. A1 runs
            # before the exp stream starts (Act idle) -> 50/50; during
            # A2/A3 Act carries the early exps -> 1/3 Act.
            epi_state = {"cycle": ["a", "v"], "i": 0}

            def epilogue(dst, ps, bias_ap):
                cyc = epi_state["cycle"]
                e = cyc[epi_state["i"] % len(cyc)]
                epi_state["i"] += 1
                if e == "a":
                    nc.scalar.activation(dst, ps, Act.Identity,
                                         bias=(bias_ap if bias_ap is not None
                                               else 0.0),
                                         scale=4.0 / WS)
                else:
                    if bias_ap is not None:
                        nc.vector.tensor_scalar(dst, ps, 4.0 / WS, bias_ap,
                                                Alu.mult, Alu.add)
                    else:
                        nc.vector.tensor_scalar(dst, ps, 4.0 / WS, None,
                                                Alu.mult)

            q8vv = q8T_sb.rearrange("p (a x) -> p a x", x=S)
            v8vv = v8T_sb.rearrange("p (a x) -> p a x", x=SEQ)
            k8vv = k8T_sb.rearrange("p (a x) -> p a x", x=SEQ)
            wvb8v = wvb8_sb.rearrange("p (a x) -> p a x", x=HEADS * D)
            oc8v = onescol8.rearrange("p (a x) -> p a x", x=128)
            on8v = ones8.rearrange("p (a x) -> p a x", x=128)
            dTv = dT8.rearrange("p (a x) -> p a x", x=2048)
            owvv = ow_sb.rearrange("p (a x) -> p a x", x=D)
            qpv8 = [qp8[h].rearrange("p (a x) -> p a x", x=S)
                    for h in range(HEADS)]

            # merged scores tile: head m's full 512 keys x the slice's 256
            # l-cols, in a 2-bank [128,1024] psum tile (blocks = 4 key
            # tiles). exp+rsub run as single 1024-col ops.
            rsub_i = [0]

            def emit_merged(ls, m, pend, attnv):
                kpv = kp8[m].rearrange("p (a x) -> p a x", x=SEQ)
                qpv = qpv8[ls]
                psp = psc.tile([128, 1024], f32, tag="sc",
                               name=f"sc_{ls}_{m}")
                for tt in range(4):
                    for dtp in range(2):
                        nc.tensor.matmul(
                            psp[:, tt * 256:(tt + 1) * 256],
                            lhsT=kpv[:, 2 * dtp:2 * dtp + 2,
                                     tt * 128:(tt + 1) * 128],
                            rhs=qpv[:, 2 * dtp:2 * dtp + 2, :],
                            start=(dtp == 0), stop=(dtp == 1),
                            perf_mode=DR)
                # pend holds half-tiles (kt2 granularity) so drained attnv
                # bursts between scores stay short (~270ns) and never
                # head-of-line-block the scores feeding the Act exp stream
                if m == 7:
                    # last tile of the slice: split exp/rsub into 512-col
                    # halves (both on DVE) so the psz z-chain launches
                    # ~1us earlier at the boundary
                    for half in range(2):
                        exh = ep.tile([128, 512], b16, tag="exh",
                                      name=f"exh_{ls}_{half}")
                        nc.scalar.activation(
                            exh, psp[:, half * 512:(half + 1) * 512],
                            Act.Exp, bias=0.0, scale=inv_sqrt_d / 16.0)
                        r8h = rp.tile([128, 512], f8, tag="r8h",
                                      name=f"r8h_{ls}_{half}")
                        nc.vector.tensor_scalar(r8h, exh, -1.0, 8.0,
                                                Alu.add, Alu.mult)
                        pend.append(
                            (m, half,
                             r8h.rearrange("p (a x) -> p a x", x=256)))
                else:
                    ex = ep.tile([128, 1024], b16, tag="ex",
                                 name=f"ex_{ls}_{m}")
                    nc.scalar.activation(ex, psp, Act.Exp, bias=0.0,
                                         scale=inv_sqrt_d / 16.0)
                    r8 = rp.tile([128, 1024], f8, tag="r8",
                                 name=f"r8_{ls}_{m}")
                    # r-sub: SBUF-only -> Pool early-m, DVE late-m (so
                    # boundary dT8 extracts aren't queued behind them)
                    rsub_i[0] += 1
                    reng = nc.vector if (ls >= 2 and m in (4, 5, 6)) \
                        else nc.gpsimd
                    reng.tensor_scalar(r8, ex, -1.0, 8.0, Alu.add, Alu.mult)
                    r8v = r8.rearrange("p (a x) -> p a x", x=256)
                    pend.append((m, 0, r8v[:, 0:2, :]))
                    pend.append((m, 1, r8v[:, 2:4, :]))
                if attnv is not None:
                    if ls == 7 and m >= 5:
                        skew = 0
                    else:
                        skew = {4: 6, 5: 6, 6: 4, 7: 2}.get(m, 99)
                    while len(pend) > skew:
                        attnv(*pend.pop(0))

            # ---------------- phase A ----------------
            pend0, pend1 = [], []
            with tc.tile_pool(name="pa2", bufs=4, space="PSUM") as pa2:
                epi_state["cycle"] = ["a", "v"]
                for h in range(HEADS):          # A1: qpT
                    for dtj in range(DT):
                        jt = h * DT + dtj
                        wkq = wk_q[jt // 8].rearrange("p (a x) -> p a x",
                                                      x=1024)
                        jo = (jt % 8) * 128
                        ps = pa2.tile([128, 512], f32, tag="asc",
                                      name=f"a1_{jt}")
                        for dtp in range(2):
                            nc.tensor.matmul(
                                ps[:, 0:S],
                                lhsT=wkq[:, 2 * dtp:2 * dtp + 2, jo:jo + 128],
                                rhs=q8vv[:, 2 * dtp:2 * dtp + 2, :],
                                start=(dtp == 0), stop=(dtp == 1),
                                perf_mode=DR)
                        epilogue(qp8[h][:, dtj * S:(dtj + 1) * S],
                                 ps[:, 0:S], wkb_sb[:, jt:jt + 1])
                epi_state["cycle"] = ["v", "a", "v"]
                for g in range(HEADS):          # A2 kp per head
                    for dtj in range(DT):
                        jt = g * DT + dtj
                        wqq = wq_q[jt // 8].rearrange("p (a x) -> p a x",
                                                      x=1024)
                        jo = (jt % 8) * 128
                        ps = pa2.tile([128, 512], f32, tag="asc",
                                      name=f"a2_{jt}")
                        for dtp in range(2):
                            nc.tensor.matmul(
                                ps,
                                lhsT=wqq[:, 2 * dtp:2 * dtp + 2, jo:jo + 128],
                                rhs=k8vv[:, 2 * dtp:2 * dtp + 2, :],
                                start=(dtp == 0), stop=(dtp == 1),
                                perf_mode=DR)
                        epilogue(kp8[g][:, dtj * SEQ:(dtj + 1) * SEQ], ps,
                                 wqb_sb[:, jt:jt + 1])
                    emit_merged(0, g, pend0, None)
                for g in range(HEADS):          # A3 vp per head
                    wvq = wv_q[g // 2].rearrange("p (a x) -> p a x", x=1024)
                    go = (g % 2) * 512
                    for tt in range(DT):
                        ps = pa2.tile([128, 512], f32, tag="asc",
                                      name=f"a3_{g}_{tt}")
                        for dtp in range(2):
                            nc.tensor.matmul(
                                ps,
                                lhsT=v8vv[:, 2 * dtp:2 * dtp + 2,
                                          tt * 128:(tt + 1) * 128],
                                rhs=wvq[:, 2 * dtp:2 * dtp + 2, go:go + 512],
                                start=(dtp == 0), stop=False, perf_mode=DR)
                        nc.tensor.matmul(            # fold 64*wv_b
                            ps, lhsT=oc8v[:, 0:2, :],
                            rhs=wvb8v[:, 0:2, g * 512:(g + 1) * 512],
                            start=False, stop=True, perf_mode=DR)
                        epilogue(vp8[g][:, tt * 512:(tt + 1) * 512], ps, None)
                    if g % 2 == 1:              # slice-1 scores m=0..3
                        emit_merged(1, g // 2, pend1, None)

            # ---------------- phase B ----------------
            with tc.tile_pool(name="psb", bufs=1, space="PSUM") as psb:
                po = psb.tile([128, 1024], f32, tag="po")
                pszt = psb.tile([128, 512], f32, tag="psz")
                psz = pszt[:, 0:256]
                psc0 = psb.tile([128, 512], f32, tag="pso")

                def make_attnv(ls):
                    def attnv(m, kt2, rhs):
                        # rhs: [128, 2, 256] r8 view for this kt2 half.
                        # Z matmul first: psz closes earlier at the slice
                        # boundary, unblocking the DVE z-chain.
                        vpg = vp8[m].rearrange("p (a x) -> p a x", x=512)
                        first = (m == 0 and kt2 == 0)
                        nc.tensor.matmul(
                            psz, lhsT=on8v[:, 0:2, :], rhs=rhs,
                            start=first,
                            stop=(m == 7 and kt2 == 1),
                            perf_mode=DR, skip_group_check=True)
                        for et in range(DT):
                            nc.tensor.matmul(
                                po[:, et * 256:(et + 1) * 256],
                                lhsT=vpg[:, 2 * kt2:2 * kt2 + 2,
                                         et * 128:(et + 1) * 128],
                                rhs=rhs,
                                start=(first and et in (0, 2)),
                                stop=False, perf_mode=DR,
                                skip_group_check=True)
                    return attnv

                def zchain(ls):
                    # -Z fold row + 1/Z, all DVE (psz readers)
                    nc.vector.tensor_scalar(
                        zfold[0:32, 0:256], psz[0:32, :], -1.0 / 8.0,
                        -4096.0, Alu.mult, Alu.add)
                    t1 = zp.tile([128, 256], f32, tag="t1", name=f"t1_{ls}")
                    nc.vector.tensor_scalar(t1, psz, 1.0 / 512.0, 64.0,
                                            Alu.mult, Alu.add)
                    zr = zp.tile([128, 256], f32, tag="zr", name=f"zr_{ls}")
                    nc.vector.reciprocal(zr, t1)
                    return zr

                def folds(ls):
                    for et in range(DT):
                        nc.tensor.matmul(
                            po[:, et * 256:(et + 1) * 256],
                            lhsT=csob[:, et * 128:(et + 1) * 128],
                            rhs=zfold[:, 0:256], start=False, stop=True,
                            skip_group_check=True)

                def extract(ls, zr):
                    for et in range(DT):
                        nc.vector.tensor_tensor(
                            out=dT8[:, et * 2048 + ls * 256:
                                    et * 2048 + (ls + 1) * 256],
                            in0=po[:, et * 256:(et + 1) * 256], in1=zr,
                            op=Alu.mult)

                def outproj0(ls):
                    # early out-projection, output rows 0:128 (st=0)
                    for etp in range(2):
                        nc.tensor.matmul(
                            psc0,
                            lhsT=dTv[:, 2 * etp:2 * etp + 2,
                                     ls * 256:ls * 256 + 128],
                            rhs=owvv[:, ls * DT + 2 * etp:
                                     ls * DT + 2 * etp + 2, :],
                            start=(ls == 0 and etp == 0),
                            stop=(ls == 7 and etp == 1),
                            perf_mode=DR, skip_group_check=True)

                def transition(ls, pend, attnv, nexts):
                    # slice-ls boundary with the next slice's first score
                    # tiles interleaved so neither PE nor Act is ever
                    # queued head-of-line behind the boundary chain.
                    # outproj0(ls-1) runs here: dT8(ls-1) is long done, so
                    # it can never stall PE on the extract chain.
                    nx = list(nexts)
                    if nx:
                        nx.pop(0)()         # feed Act before the flush
                    cnt = 0
                    while len(pend) > 2:
                        attnv(*pend.pop(0))
                        cnt += 1
                        if cnt % 4 == 0 and nx:
                            nx.pop(0)()
                    if nx:
                        nx.pop(0)()
                    for args in pend:       # last halves close psz
                        attnv(*args)
                    pend.clear()
                    if ls > 0:
                        outproj0(ls - 1)
                    zr = zchain(ls)
                    if nx:
                        nx.pop(0)()
                    folds(ls)
                    if nx:
                        nx.pop(0)()
                    extract(ls, zr)
                    while nx:
                        nx.pop(0)()

                def em(ls, m, pend, attnv=None):
                    return lambda: emit_merged(ls, m, pend, attnv)

                atts = [make_attnv(ls) for ls in range(2 * 4)]
                pend2 = []
                transition(0, pend0, atts[0],
                           [em(1, 4, pend1), em(1, 5, pend1),
                            em(1, 6, pend1), em(1, 7, pend1)])
                transition(1, pend1, atts[1],
                           [em(2, 0, pend2), em(2, 1, pend2),
                            em(2, 2, pend2), em(2, 3, pend2)])
                pend = pend2
                for ls in range(2, 8):
                    for m in range(4 if ls == 2 else 2, 8):
                        emit_merged(ls, m, pend, atts[ls])
                    if ls < 7:
                        pend_next = []
                        transition(ls, pend, atts[ls],
                                   [em(ls + 1, 0, pend_next),
                                    em(ls + 1, 1, pend_next)])
                        pend = pend_next
                    else:
                        transition(ls, pend, atts[ls], [])

                # tail: out rows 128:256 (st=1) into the freed psz bank.
                # hh 0..6 depend only on already-extracted dT8 slices, so
                # they run while the slice-7 extract chain drains; the
                # final outproj0(7) (waiting on extract) comes after.
                def st1(h0, h1):
                    for hh in range(h0, h1):
                        for etp in range(2):
                            nc.tensor.matmul(
                                pszt,
                                lhsT=dTv[:, 2 * etp:2 * etp + 2,
                                         hh * 256 + 128:hh * 256 + 256],
                                rhs=owvv[:, hh * DT + 2 * etp:
                                         hh * DT + 2 * etp + 2, :],
                                start=(hh == 0 and etp == 0),
                                stop=(hh == 7 and etp == 1),
                                perf_mode=DR, skip_group_check=True)

                st1(0, 7)
                outproj0(7)
                st1(7, 8)
                nc.scalar.activation(
                    tmp_sb[:, 0:D], psc0,
                    Act.Identity, bias=0.0, scale=1.0 / (2048.0 * WS))
                nc.vector.tensor_tensor(
                    out=out_sb[:, 0:D], in0=tmp_sb[:, 0:D], in1=b_eff,
                    op=Alu.add)
                nc.sync.dma_start(out=out[0:128, :], in_=out_sb[:, 0:D])
                nc.scalar.activation(
                    tmp_sb[:, D:2 * D], pszt,
                    Act.Identity, bias=0.0, scale=1.0 / (2048.0 * WS))
                nc.vector.tensor_tensor(
                    out=out_sb[:, D:2 * D], in0=tmp_sb[:, D:2 * D],
                    in1=b_eff, op=Alu.add)
                nc.sync.dma_start(out=out[128:256, :],
                                  in_=out_sb[:, D:2 * D])

    nc.compile()
    return nc


def _get_program():
    if "nc" not in _CACHE:
        _CACHE["nc"] = _build_program()
    return _CACHE["nc"]


def _prep_shared(inputs):
    f8 = NP_F8
    c = np.ascontiguousarray
    f32 = np.float32

    def t8(x, scale=1.0):
        return c((np.asarray(x, f32) * scale).T).astype(f8)

    # bias fold: sum_k (1/16) * (8*wv_b) over 128 partitions = 64*wv_b.
    # 1/16 stays fp8-normal (1/128 would be subnormal -> FTZ risk).
    wvb = np.zeros((128, 2 * HEADS * D), f32)
    wvb[:, :HEADS * D] = np.asarray(inputs["wv_b"], f32)[None, :] * 8.0
    onescol = np.zeros((128, 256), f32)
    onescol[:, :128] = 1.0 / 16.0
    zfold = np.zeros((64, SEQ), f32)
    zfold[32, :] = 1.0
    return {
        "wk8T": t8(inputs["wk_w"], WS),
        "wq8T": t8(inputs["wq_w"], WS),
        "wv8T": t8(inputs["wv_w"], WS),
        "ow8T": t8(inputs["out_w"], WS),
        # biases x4: projections are stored 4x-scaled in fp8 (subnormal
        # avoidance); epilogue computes ps*4/WS + 4*b.
        "wkb": c(np.asarray(inputs["wk_b"], f32).reshape(JT, 128).T) * 4.0,
        "wqb": c(np.asarray(inputs["wq_b"], f32).reshape(JT, 128).T) * 4.0,
        "wvb8": wvb.astype(f8),
        "ones8": np.ones((128, 256), f8),
        "onescol8": onescol.astype(f8),
        "zfoldi": zfold.astype(NP_BF16),
    }


def _make_in_maps(inputs):
    f8 = NP_F8
    c = np.ascontiguousarray
    shared = _prep_shared(inputs)
    q = np.asarray(inputs["q"], np.float32)
    k = np.asarray(inputs["k"], np.float32)
    v = np.asarray(inputs["v"], np.float32)
    wv_w = np.asarray(inputs["wv_w"], np.float64)
    wv_b = np.asarray(inputs["wv_b"], np.float64)
    ow = np.asarray(inputs["out_w"], np.float64)
    ob = np.asarray(inputs["out_b"], np.float64)

    per_batch = []
    for b in range(BS):
        vsum = v[b].astype(np.float64).sum(axis=0)
        colsum = (vsum @ wv_w.T + SEQ * wv_b).reshape(HEADS, D).sum(axis=0)
        cs_bf = colsum.astype(NP_BF16)
        obar_bf = (colsum / (SEQ * HEADS)).astype(NP_BF16)
        # x32: attn psum is at scale 8(r) * 4(vp) = 32
        csob = np.zeros((64, SEQ), np.float32)
        csob[0, :] = obar_bf.astype(np.float32) * 32.0
        csob[32, :] = cs_bf.astype(np.float32) * 32.0
        b_eff = (np.tile(obar_bf.astype(np.float64), HEADS) @ ow.T + ob
                 ).astype(np.float32)
        per_batch.append({
            "k8T": c(k[b].T).astype(f8),
            "v8T": c(v[b].T).astype(f8),
            "csob": csob.astype(NP_BF16),
            "b_eff": np.broadcast_to(b_eff[None, :], (128, D)).copy(),
        })

    in_maps = []
    for core in range(NCORES):
        b, half = divmod(core, 2)
        m = dict(shared)
        m.update(per_batch[b])
        m["q8T"] = c(q[b, half * S:(half + 1) * S, :].T).astype(f8)
        in_maps.append(m)
    return in_maps


def kernel(**inputs):
    from concourse.bass_utils import run_bass_kernel_spmd

    nc = _get_program()
    in_maps = _make_in_maps(inputs)
    res = run_bass_kernel_spmd(nc, in_maps, core_ids=list(range(NCORES)))
    _CACHE["last_results"] = res
    out = np.empty((BS, SEQ, D), np.float32)
    for core in range(NCORES):
        b, half = divmod(core, 2)
        out[b, half * S:(half + 1) * S, :] = res.results[core]["out"]
    return out


if __name__ == "__main__":
    rng = np.random.default_rng(0)
    fake = {
        "q": rng.standard_normal((BS, SEQ, D)).astype(np.float32),
        "k": rng.standard_normal((BS, SEQ, D)).astype(np.float32),
        "v": rng.standard_normal((BS, SEQ, D)).astype(np.float32),
        "wq_w": (rng.standard_normal((D * HEADS, D)) * 0.02).astype(np.float32),
        "wq_b": (rng.standard_normal((D * HEADS,)) * 0.02).astype(np.float32),
        "wk_w": (rng.standard_normal((D * HEADS, D)) * 0.02).astype(np.float32),
        "wk_b": (rng.standard_normal((D * HEADS,)) * 0.02).astype(np.float32),
        "wv_w": (rng.standard_normal((D * HEADS, D)) * 0.02).astype(np.float32),
        "wv_b": (rng.standard_normal((D * HEADS,)) * 0.02).astype(np.float32),
        "out_w": (rng.standard_normal((D, D * HEADS)) * 0.02).astype(np.float32),
        "out_b": (rng.standard_normal((D,)) * 0.02).astype(np.float32),
    }
    o = kernel(**fake)
    print("kernel ran, out shape", o.shape, "std", o.std())


# revision 20
# speedup vs baseline: 1.0611x; 1.0244x over previous
"""Trainium2 Bass kernel for nn_MultiHeadAttention_48086453846410 (fp8).

Reference (heads folded into seq axis, softmax over FULL L = seq*heads keys):
    qp = (q @ wk_w.T + wk_b).reshape(bs, L, d)   # swapped wk/wq, faithful
    kp = (k @ wq_w.T + wq_b).reshape(bs, L, d)
    vp = (v @ wv_w.T + wv_b).reshape(bs, L, d)
    scores = qp @ kp.T / sqrt(d); attn = softmax(scores, -1)
    out = (attn @ vp).reshape(bs, seq, d*heads) @ out_w.T + out_b

Sharding: 8 cores = (batch b) x (seq half). Each core owns 256 query rows
(2048 l-rows), softmax over keys -> no collectives.

Speed strategy (cost model): fp8e4 DoubleRow matmuls (0.5 cyc/out-col).
Elementwise work spread over Act/DVE/Pool with Act reserved for the exp
stream in phase B (exp only runs on Act). Scores accumulate in 2-bank
[128,1024] psum tiles so exp/r-sub run as 1024-col ops (amortizes the
~370ns Act access-latency per instruction). po (attn@v accum) is single-
buffered; accumulation groups open via start=True on the first write per
bank (lazy zero-region semantics), no zero matmuls. Out-projection for
the first 128 output rows accumulates per-slice into a spare psum bank.

Precision strategy (gate 2e-2; baseline measured 0.0122 on HW):
 - weights scaled x64 on host before fp8 cast (w std 0.02 is subnormal).
 - attn weights: r = exp(s) - 1 (std 0.2) matmul'd in fp8; the "1 * vp"
   mean flow restored exactly via host-computed colsum through a bf16
   fold matmul: po' = po + colsum*1 - obar*Z.
 - out-projection mean-extraction: delta = (o - obar) in fp8; mean path
   obar @ ow.T + out_b = b_eff host-exact.  Z = 4096 + sum(r) from DR
   ones-matmuls (exact fp32 psum).
 - fp8 scales: weights x64, projections x4, r x8, delta x2048/Z.

Structure: phase A projections (4-bank rotating psum pool, epilogues on
DVE/Act); slice-0 scores emitted behind A2 heads, slice-1 m=0..3 behind
A3 heads. Phase B: 8 slices (one head's 256 l-cols each) of 8 merged
score tiles (512 keys each): 8 DR matmuls -> exp [128,1024] (Act) ->
r-sub (Pool/DVE) -> deferred attnv+Z DR matmuls. Slice boundary: -Z fold
(DVE) -> csob fold matmuls -> 1/Z (DVE) -> dT8 delta extract (DVE) ->
early out-proj (st=0). Tail: st=1 out-proj into the freed psz bank.
"""

import math
import sys

for _p in ("/opt/trn_rl_repo",):
    if _p not in sys.path:
        sys.path.insert(0, _p)

import numpy as np
import ml_dtypes

BS, SEQ, D, HEADS = 4, 512, 512, 8
NCORES = 8
S = SEQ // 2            # 256 query seq rows per core
JT = HEADS * D // 128   # 32 j-tiles of the 4096 projection dim
DT = D // 128           # 4 d-tiles of the 512 contraction dim
WS = 64.0               # host fp8 weight scale
NP_BF16 = ml_dtypes.bfloat16
NP_F8 = ml_dtypes.float8_e4m3

_CACHE = {}


def _build_program():
    from concourse import bacc
    import concourse.mybir as mybir
    import concourse.tile as tile
    from concourse.dt import dt

    f32 = dt.float32
    b16 = dt.bfloat16
    f8 = dt.float8e4
    Act = mybir.ActivationFunctionType
    Alu = mybir.AluOpType
    DR = mybir.MatmulPerfMode.DoubleRow

    nc = bacc.Bacc(None, target_bir_lowering=False, debug=False,
                   num_devices=NCORES)

    def din(name, shape, dty):
        return nc.dram_tensor(name, shape, dty, kind="ExternalInput").ap()

    q8T = din("q8T", [D, S], f8)                # q[b,half].T  (d, s)
    k8T = din("k8T", [D, SEQ], f8)              # k[b].T       (d, t)
    v8T = din("v8T", [D, SEQ], f8)              # v[b].T       (d, t)
    wk8T = din("wk8T", [D, HEADS * D], f8)      # 64*wk_w.T    (d, j)
    wq8T = din("wq8T", [D, HEADS * D], f8)
    wv8T = din("wv8T", [D, HEADS * D], f8)
    ow8T = din("ow8T", [HEADS * D, D], f8)      # 64*out_w.T   (c, r)
    wkb = din("wkb", [128, JT], f32)            # wk_b.reshape(JT,128).T
    wqb = din("wqb", [128, JT], f32)
    wvb8 = din("wvb8", [128, 2 * HEADS * D], f8)  # [64*wv_b repl | zeros]
    ones8d = din("ones8", [128, 256], f8)       # DR ones (Z matmul lhsT)
    onescol8d = din("onescol8", [128, 256], f8)  # [1/16 | 0] bias-fold lhsT
    csobd = din("csob", [64, SEQ], b16)    # r0=obar r32=colsum rest 0
    zfoldd = din("zfoldi", [64, SEQ], b16)      # r32=ones rest 0
    b_effd = din("b_eff", [128, D], f32)        # obar@owT+out_b replicated
    out = nc.dram_tensor("out", [S, D], f32, kind="ExternalOutput").ap()

    inv_sqrt_d = 1.0 / math.sqrt(D)

    with tile.TileContext(nc) as tc:
        with (
            tc.tile_pool(name="big", bufs=1) as bp,
            tc.tile_pool(name="exp", bufs=6) as ep,
            tc.tile_pool(name="r8p", bufs=14) as rp,
            tc.tile_pool(name="zrp", bufs=2) as zp,
            tc.tile_pool(name="psc", bufs=2, space="PSUM") as psc,
        ):
            # ---------------- DMAs (first-needed-first) ----------------
            def dma(dst, src):
                nc.sync.dma_start(out=dst, in_=src)

            q8T_sb = bp.tile([128, DT * S], f8, tag="q8T")
            dma(q8T_sb.rearrange("p (t n) -> p t n", n=S),
                q8T.rearrange("(t p) n -> p t n", p=128))
            wkd = wk8T.rearrange("(t p) n -> p t n", p=128)
            wk_q = []
            for c in range(4):
                t = bp.tile([128, DT * 1024], f8, tag=f"wk{c}",
                            name=f"wk{c}")
                tv = t.rearrange("p (t n) -> p t n", n=1024)
                for hc in range(2):
                    dma(tv[:, :, hc * 512:(hc + 1) * 512],
                        wkd[:, :, c * 1024 + hc * 512:
                             c * 1024 + (hc + 1) * 512])
                wk_q.append(t)
                if c == 0:
                    wkb_sb = bp.tile([128, JT], f32, tag="wkb")
                    dma(wkb_sb, wkb)
            k8T_sb = bp.tile([128, DT * SEQ], f8, tag="k8T")
            dma(k8T_sb.rearrange("p (t n) -> p t n", n=SEQ),
                k8T.rearrange("(t p) n -> p t n", p=128))
            wqb_sb = bp.tile([128, JT], f32, tag="wqb")
            dma(wqb_sb, wqb)
            wqd = wq8T.rearrange("(t p) n -> p t n", p=128)
            wq_q = []
            for c in range(4):
                t = bp.tile([128, DT * 1024], f8, tag=f"wq{c}",
                            name=f"wq{c}")
                dma(t.rearrange("p (t n) -> p t n", n=1024),
                    wqd[:, :, c * 1024:(c + 1) * 1024])
                wq_q.append(t)
            v8T_sb = bp.tile([128, DT * SEQ], f8, tag="v8T")
            dma(v8T_sb.rearrange("p (t n) -> p t n", n=SEQ),
                v8T.rearrange("(t p) n -> p t n", p=128))
            wvb8_sb = bp.tile([128, 2 * HEADS * D], f8, tag="wvb8")
            dma(wvb8_sb, wvb8)
            onescol8 = bp.tile([128, 256], f8, tag="onescol8")
            dma(onescol8, onescol8d)
            ones8 = bp.tile([128, 256], f8, tag="ones8")
            dma(ones8, ones8d)
            wvd = wv8T.rearrange("(t p) n -> p t n", p=128)
            wv_q = []
            for c in range(4):
                t = bp.tile([128, DT * 1024], f8, tag=f"wv{c}",
                            name=f"wv{c}")
                dma(t.rearrange("p (t n) -> p t n", n=1024),
                    wvd[:, :, c * 1024:(c + 1) * 1024])
                wv_q.append(t)
            csob = bp.tile([64, SEQ], b16, tag="csob")
            dma(csob, csobd)
            zfold = bp.tile([64, SEQ], b16, tag="zfold")
            dma(zfold, zfoldd)
            ow_sb = bp.tile([128, JT * D], f8, tag="ow")
            owv = ow_sb.rearrange("p (t n) -> p t n", n=D)
            owd = ow8T.rearrange("(t p) n -> p t n", p=128)
            for c in range(4):
                dma(owv[:, 8 * c:8 * (c + 1), :],
                    owd[:, 8 * c:8 * (c + 1), :])
            b_eff = bp.tile([128, D], f32, tag="beff")
            dma(b_eff, b_effd)

            # ---------------- persistent SBUF state ----------------
            qp8 = [bp.tile([128, DT * S], f8, tag=f"qp{h}", name=f"qp{h}")
                   for h in range(HEADS)]          # cols dtj*S + s
            kp8 = [bp.tile([128, DT * SEQ], f8, tag=f"kp{g}", name=f"kp{g}")
                   for g in range(HEADS)]          # cols dt*SEQ + t
            vp8 = [bp.tile([128, DT * 512], f8, tag=f"vg{g}", name=f"vg{g}")
                   for g in range(HEADS)]          # cols tt*512 + e
            dT8 = bp.tile([128, DT * 2048], f8, tag="dT8")      # et*2048+l
            out_sb = bp.tile([128, 2 * D], f32, tag="outsb")
            tmp_sb = bp.tile([128, 2 * D], f32, tag="tmpsb")

            # phase-A epilogues: psum readers -> DVE or Act only. A1 runs
            # before the exp stream starts (Act idle) -> 50/50; during
            # A2/A3 Act carries the early exps -> 1/3 Act.
            epi_state = {"cycle": ["a", "v"], "i": 0}

            def epilogue(dst, ps, bias_ap):
                cyc = epi_state["cycle"]
                e = cyc[epi_state["i"] % len(cyc)]
                epi_state["i"] += 1
                if e == "a":
                    nc.scalar.activation(dst, ps, Act.Identity,
                                         bias=(bias_ap if bias_ap is not None
                                               else 0.0),
                                         scale=4.0 / WS)
                else:
                    if bias_ap is not None:
                        nc.vector.tensor_scalar(dst, ps, 4.0 / WS, bias_ap,
                                                Alu.mult, Alu.add)
                    else:
                        nc.vector.tensor_scalar(dst, ps, 4.0 / WS, None,
                                                Alu.mult)

            q8vv = q8T_sb.rearrange("p (a x) -> p a x", x=S)
            v8vv = v8T_sb.rearrange("p (a x) -> p a x", x=SEQ)
            k8vv = k8T_sb.rearrange("p (a x) -> p a x", x=SEQ)
            wvb8v = wvb8_sb.rearrange("p (a x) -> p a x", x=HEADS * D)
            oc8v = onescol8.rearrange("p (a x) -> p a x", x=128)
            on8v = ones8.rearrange("p (a x) -> p a x", x=128)
            dTv = dT8.rearrange("p (a x) -> p a x", x=2048)
            owvv = ow_sb.rearrange("p (a x) -> p a x", x=D)
            qpv8 = [qp8[h].rearrange("p (a x) -> p a x", x=S)
                    for h in range(HEADS)]

            # merged scores tile: head m's full 512 keys x the slice's 256
            # l-cols, in a 2-bank [128,1024] psum tile (blocks = 4 key
            # tiles). exp+rsub run as single 1024-col ops.
            rsub_i = [0]

            def emit_merged(ls, m, pend, attnv):
                kpv = kp8[m].rearrange("p (a x) -> p a x", x=SEQ)
                qpv = qpv8[ls]
                psp = psc.tile([128, 1024], f32, tag="sc",
                               name=f"sc_{ls}_{m}")
                for tt in range(4):
                    for dtp in range(2):
                        nc.tensor.matmul(
                            psp[:, tt * 256:(tt + 1) * 256],
                            lhsT=kpv[:, 2 * dtp:2 * dtp + 2,
                                     tt * 128:(tt + 1) * 128],
                            rhs=qpv[:, 2 * dtp:2 * dtp + 2, :],
                            start=(dtp == 0), stop=(dtp == 1),
                            perf_mode=DR)
                # pend holds half-tiles (kt2 granularity) so drained attnv
                # bursts between scores stay short (~270ns) and never
                # head-of-line-block the scores feeding the Act exp stream
                if m == 7:
                    # last tile of the slice: split exp/rsub into 512-col
                    # halves (both on DVE) so the psz z-chain launches
                    # ~1us earlier at the boundary
                    for half in range(2):
                        exh = ep.tile([128, 512], b16, tag="exh",
                                      name=f"exh_{ls}_{half}")
                        nc.scalar.activation(
                            exh, psp[:, half * 512:(half + 1) * 512],
                            Act.Exp, bias=0.0, scale=inv_sqrt_d / 16.0)
                        r8h = rp.tile([128, 512], f8, tag="r8h",
                                      name=f"r8h_{ls}_{half}")
                        nc.vector.tensor_scalar(r8h, exh, -1.0, 8.0,
                                                Alu.add, Alu.mult)
                        pend.append(
                            (m, half,
                             r8h.rearrange("p (a x) -> p a x", x=256)))
                else:
                    ex = ep.tile([128, 1024], b16, tag="ex",
                                 name=f"ex_{ls}_{m}")
                    nc.scalar.activation(ex, psp, Act.Exp, bias=0.0,
                                         scale=inv_sqrt_d / 16.0)
                    r8 = rp.tile([128, 1024], f8, tag="r8",
                                 name=f"r8_{ls}_{m}")
                    # r-sub: SBUF-only -> Pool early-m, DVE late-m (so
                    # boundary dT8 extracts aren't queued behind them)
                    rsub_i[0] += 1
                    reng = nc.vector if (ls >= 2 and m in (4, 5, 6)) \
                        else nc.gpsimd
                    reng.tensor_scalar(r8, ex, -1.0, 8.0, Alu.add, Alu.mult)
                    r8v = r8.rearrange("p (a x) -> p a x", x=256)
                    pend.append((m, 0, r8v[:, 0:2, :]))
                    pend.append((m, 1, r8v[:, 2:4, :]))
                if attnv is not None:
                    if ls == 7 and m >= 5:
                        skew = 0
                    else:
                        skew = {6: 5, 7: 4}.get(m, 6)
                    # cap drain bursts at 2 halves (~530ns) so they never
                    # delay the next scores tile past Act's 1038ns cadence
                    pops = 0
                    while len(pend) > skew and pops < 2:
                        attnv(*pend.pop(0))
                        pops += 1

            # ---------------- phase A ----------------
            pend0, pend1 = [], []
            with tc.tile_pool(name="pa2", bufs=4, space="PSUM") as pa2:
                epi_state["cycle"] = ["a", "v"]
                for h in range(HEADS):          # A1: qpT
                    for dtj in range(DT):
                        jt = h * DT + dtj
                        wkq = wk_q[jt // 8].rearrange("p (a x) -> p a x",
                                                      x=1024)
                        jo = (jt % 8) * 128
                        ps = pa2.tile([128, 512], f32, tag="asc",
                                      name=f"a1_{jt}")
                        for dtp in range(2):
                            nc.tensor.matmul(
                                ps[:, 0:S],
                                lhsT=wkq[:, 2 * dtp:2 * dtp + 2, jo:jo + 128],
                                rhs=q8vv[:, 2 * dtp:2 * dtp + 2, :],
                                start=(dtp == 0), stop=(dtp == 1),
                                perf_mode=DR)
                        epilogue(qp8[h][:, dtj * S:(dtj + 1) * S],
                                 ps[:, 0:S], wkb_sb[:, jt:jt + 1])
                epi_state["cycle"] = ["v", "a", "v"]
                for g in range(HEADS):          # A2 kp per head
                    for dtj in range(DT):
                        jt = g * DT + dtj
                        wqq = wq_q[jt // 8].rearrange("p (a x) -> p a x",
                                                      x=1024)
                        jo = (jt % 8) * 128
                        ps = pa2.tile([128, 512], f32, tag="asc",
                                      name=f"a2_{jt}")
                        for dtp in range(2):
                            nc.tensor.matmul(
                                ps,
                                lhsT=wqq[:, 2 * dtp:2 * dtp + 2, jo:jo + 128],
                                rhs=k8vv[:, 2 * dtp:2 * dtp + 2, :],
                                start=(dtp == 0), stop=(dtp == 1),
                                perf_mode=DR)
                        epilogue(kp8[g][:, dtj * SEQ:(dtj + 1) * SEQ], ps,
                                 wqb_sb[:, jt:jt + 1])
                    emit_merged(0, g, pend0, None)
                for g in range(HEADS):          # A3 vp per head
                    wvq = wv_q[g // 2].rearrange("p (a x) -> p a x", x=1024)
                    go = (g % 2) * 512
                    for tt in range(DT):
                        ps = pa2.tile([128, 512], f32, tag="asc",
                                      name=f"a3_{g}_{tt}")
                        for dtp in range(2):
                            nc.tensor.matmul(
                                ps,
                                lhsT=v8vv[:, 2 * dtp:2 * dtp + 2,
                                          tt * 128:(tt + 1) * 128],
                                rhs=wvq[:, 2 * dtp:2 * dtp + 2, go:go + 512],
                                start=(dtp == 0), stop=False, perf_mode=DR)
                        nc.tensor.matmul(            # fold 64*wv_b
                            ps, lhsT=oc8v[:, 0:2, :],
                            rhs=wvb8v[:, 0:2, g * 512:(g + 1) * 512],
                            start=False, stop=True, perf_mode=DR)
                        epilogue(vp8[g][:, tt * 512:(tt + 1) * 512], ps, None)
                    if g % 2 == 1:              # slice-1 scores m=0..3
                        emit_merged(1, g // 2, pend1, None)

            # ---------------- phase B ----------------
            with tc.tile_pool(name="psb", bufs=1, space="PSUM") as psb:
                po = psb.tile([128, 1024], f32, tag="po")
                pszt = psb.tile([128, 512], f32, tag="psz")
                psz = pszt[:, 0:256]
                psc0 = psb.tile([128, 512], f32, tag="pso")

                def make_attnv(ls):
                    def attnv(m, kt2, rhs):
                        # rhs: [128, 2, 256] r8 view for this kt2 half.
                        # Z matmul first: psz closes earlier at the slice
                        # boundary, unblocking the DVE z-chain.
                        vpg = vp8[m].rearrange("p (a x) -> p a x", x=512)
                        first = (m == 0 and kt2 == 0)
                        nc.tensor.matmul(
                            psz, lhsT=on8v[:, 0:2, :], rhs=rhs,
                            start=first,
                            stop=(m == 7 and kt2 == 1),
                            perf_mode=DR, skip_group_check=True)
                        for et in range(DT):
                            nc.tensor.matmul(
                                po[:, et * 256:(et + 1) * 256],
                                lhsT=vpg[:, 2 * kt2:2 * kt2 + 2,
                                         et * 128:(et + 1) * 128],
                                rhs=rhs,
                                start=(first and et in (0, 2)),
                                stop=False, perf_mode=DR,
                                skip_group_check=True)
                    return attnv

                def zchain(ls):
                    # -Z fold row + 1/Z, all DVE (psz readers)
                    nc.vector.tensor_scalar(
                        zfold[0:32, 0:256], psz[0:32, :], -1.0 / 8.0,
                        -4096.0, Alu.mult, Alu.add)
                    t1 = zp.tile([128, 256], f32, tag="t1", name=f"t1_{ls}")
                    nc.vector.tensor_scalar(t1, psz, 1.0 / 512.0, 64.0,
                                            Alu.mult, Alu.add)
                    zr = zp.tile([128, 256], f32, tag="zr", name=f"zr_{ls}")
                    nc.vector.reciprocal(zr, t1)
                    return zr

                def folds(ls):
                    for et in range(DT):
                        nc.tensor.matmul(
                            po[:, et * 256:(et + 1) * 256],
                            lhsT=csob[:, et * 128:(et + 1) * 128],
                            rhs=zfold[:, 0:256], start=False, stop=True,
                            skip_group_check=True)

                def extract(ls, zr):
                    # single merged dT8 extract: one DVE op over all 4 et
                    # blocks (po contiguous, dT8 strided, zr broadcast) --
                    # 3 fewer serial ~550ns DVE steps on the slice-boundary
                    # critical path, and next-slice attnvs gate atomically.
                    try:
                        dst = dT8.rearrange("p (a x) -> p a x",
                                            x=2048)[:, :,
                                                    ls * 256:(ls + 1) * 256]
                        po4 = po.rearrange("p (a x) -> p a x", x=256)
                        zrb = zr.rearrange("p (a x) -> p a x",
                                           x=256).broadcast_to([128, 4, 256])
                        nc.vector.tensor_tensor(out=dst, in0=po4, in1=zrb,
                                                op=Alu.mult)
                    except Exception:
                        for et in range(DT):
                            nc.vector.tensor_tensor(
                                out=dT8[:, et * 2048 + ls * 256:
                                        et * 2048 + (ls + 1) * 256],
                                in0=po[:, et * 256:(et + 1) * 256], in1=zr,
                                op=Alu.mult)

                def outproj0(ls):
                    # early out-projection, output rows 0:128 (st=0)
                    for etp in range(2):
                        nc.tensor.matmul(
                            psc0,
                            lhsT=dTv[:, 2 * etp:2 * etp + 2,
                                     ls * 256:ls * 256 + 128],
                            rhs=owvv[:, ls * DT + 2 * etp:
                                     ls * DT + 2 * etp + 2, :],
                            start=(ls == 0 and etp == 0),
                            stop=(ls == 7 and etp == 1),
                            perf_mode=DR, skip_group_check=True)

                def transition(ls, pend, attnv, nexts):
                    # slice-ls boundary with the next slice's first score
                    # tiles interleaved so neither PE nor Act is ever
                    # queued head-of-line behind the boundary chain.
                    # outproj0(ls-1) runs here: dT8(ls-1) is long done, so
                    # it can never stall PE on the extract chain.
                    nx = list(nexts)
                    if nx:
                        nx.pop(0)()         # feed Act before the flush
                    cnt = 0
                    while len(pend) > 2:
                        attnv(*pend.pop(0))
                        cnt += 1
                        if cnt % 4 == 0 and nx:
                            nx.pop(0)()
                    if nx:
                        nx.pop(0)()
                    for args in pend:       # last halves close psz
                        attnv(*args)
                    pend.clear()
                    if ls > 0:
                        outproj0(ls - 1)
                    zr = zchain(ls)
                    if nx:
                        nx.pop(0)()
                    folds(ls)
                    if nx:
                        nx.pop(0)()
                    extract(ls, zr)
                    while nx:
                        nx.pop(0)()

                def em(ls, m, pend, attnv=None):
                    return lambda: emit_merged(ls, m, pend, attnv)

                atts = [make_attnv(ls) for ls in range(2 * 4)]
                pend2 = []
                transition(0, pend0, atts[0],
                           [em(1, 4, pend1), em(1, 5, pend1),
                            em(1, 6, pend1), em(1, 7, pend1)])
                transition(1, pend1, atts[1],
                           [em(2, 0, pend2), em(2, 1, pend2),
                            em(2, 2, pend2), em(2, 3, pend2)])
                pend = pend2
                for ls in range(2, 8):
                    for m in range(4 if ls == 2 else 2, 8):
                        emit_merged(ls, m, pend, atts[ls])
                    if ls < 7:
                        pend_next = []
                        transition(ls, pend, atts[ls],
                                   [em(ls + 1, 0, pend_next),
                                    em(ls + 1, 1, pend_next)])
                        pend = pend_next
                    else:
                        transition(ls, pend, atts[ls], [])

                # tail: out rows 128:256 (st=1) into the freed psz bank.
                # hh 0..6 depend only on already-extracted dT8 slices, so
                # they run while the slice-7 extract chain drains; the
                # final outproj0(7) (waiting on extract) comes after.
                def st1(h0, h1):
                    for hh in range(h0, h1):
                        for etp in range(2):
                            nc.tensor.matmul(
                                pszt,
                                lhsT=dTv[:, 2 * etp:2 * etp + 2,
                                         hh * 256 + 128:hh * 256 + 256],
                                rhs=owvv[:, hh * DT + 2 * etp:
                                         hh * DT + 2 * etp + 2, :],
                                start=(hh == 0 and etp == 0),
                                stop=(hh == 7 and etp == 1),
                                perf_mode=DR, skip_group_check=True)

                st1(0, 7)
                outproj0(7)
                st1(7, 8)
                nc.scalar.activation(
                    tmp_sb[:, 0:D], psc0,
                    Act.Identity, bias=0.0, scale=1.0 / (2048.0 * WS))
                nc.vector.tensor_tensor(
                    out=out_sb[:, 0:D], in0=tmp_sb[:, 0:D], in1=b_eff,
                    op=Alu.add)
                nc.sync.dma_start(out=out[0:128, :], in_=out_sb[:, 0:D])
                nc.scalar.activation(
                    tmp_sb[:, D:2 * D], pszt,
                    Act.Identity, bias=0.0, scale=1.0 / (2048.0 * WS))
                nc.vector.tensor_tensor(
                    out=out_sb[:, D:2 * D], in0=tmp_sb[:, D:2 * D],
                    in1=b_eff, op=Alu.add)
                nc.sync.dma_start(out=out[128:256, :],
                                  in_=out_sb[:, D:2 * D])

    nc.compile()
    return nc


def _get_program():
    if "nc" not in _CACHE:
        _CACHE["nc"] = _build_program()
    return _CACHE["nc"]


def _prep_shared(inputs):
    f8 = NP_F8
    c = np.ascontiguousarray
    f32 = np.float32

    def t8(x, scale=1.0):
        return c((np.asarray(x, f32) * scale).T).astype(f8)

    # bias fold: sum_k (1/16) * (8*wv_b) over 128 partitions = 64*wv_b.
    # 1/16 stays fp8-normal (1/128 would be subnormal -> FTZ risk).
    wvb = np.zeros((128, 2 * HEADS * D), f32)
    wvb[:, :HEADS * D] = np.asarray(inputs["wv_b"], f32)[None, :] * 8.0
    onescol = np.zeros((128, 256), f32)
    onescol[:, :128] = 1.0 / 16.0
    zfold = np.zeros((64, SEQ), f32)
    zfold[32, :] = 1.0
    return {
        "wk8T": t8(inputs["wk_w"], WS),
        "wq8T": t8(inputs["wq_w"], WS),
        "wv8T": t8(inputs["wv_w"], WS),
        "ow8T": t8(inputs["out_w"], WS),
        # biases x4: projections are stored 4x-scaled in fp8 (subnormal
        # avoidance); epilogue computes ps*4/WS + 4*b.
        "wkb": c(np.asarray(inputs["wk_b"], f32).reshape(JT, 128).T) * 4.0,
        "wqb": c(np.asarray(inputs["wq_b"], f32).reshape(JT, 128).T) * 4.0,
        "wvb8": wvb.astype(f8),
        "ones8": np.ones((128, 256), f8),
        "onescol8": onescol.astype(f8),
        "zfoldi": zfold.astype(NP_BF16),
    }


def _make_in_maps(inputs):
    f8 = NP_F8
    c = np.ascontiguousarray
    shared = _prep_shared(inputs)
    q = np.asarray(inputs["q"], np.float32)
    k = np.asarray(inputs["k"], np.float32)
    v = np.asarray(inputs["v"], np.float32)
    wv_w = np.asarray(inputs["wv_w"], np.float64)
    wv_b = np.asarray(inputs["wv_b"], np.float64)
    ow = np.asarray(inputs["out_w"], np.float64)
    ob = np.asarray(inputs["out_b"], np.float64)

    per_batch = []
    for b in range(BS):
        vsum = v[b].astype(np.float64).sum(axis=0)
        colsum = (vsum @ wv_w.T + SEQ * wv_b).reshape(HEADS, D).sum(axis=0)
        cs_bf = colsum.astype(NP_BF16)
        obar_bf = (colsum / (SEQ * HEADS)).astype(NP_BF16)
        # x32: attn psum is at scale 8(r) * 4(vp) = 32
        csob = np.zeros((64, SEQ), np.float32)
        csob[0, :] = obar_bf.astype(np.float32) * 32.0
        csob[32, :] = cs_bf.astype(np.float32) * 32.0
        b_eff = (np.tile(obar_bf.astype(np.float64), HEADS) @ ow.T + ob
                 ).astype(np.float32)
        per_batch.append({
            "k8T": c(k[b].T).astype(f8),
            "v8T": c(v[b].T).astype(f8),
            "csob": csob.astype(NP_BF16),
            "b_eff": np.broadcast_to(b_eff[None, :], (128, D)).copy(),
        })

    in_maps = []
    for core in range(NCORES):
        b, half = divmod(core, 2)
        m = dict(shared)
        m.update(per_batch[b])
        m["q8T"] = c(q[b, half * S:(half + 1) * S, :].T).astype(f8)
        in_maps.append(m)
    return in_maps


def kernel(**inputs):
    from concourse.bass_utils import run_bass_kernel_spmd

    nc = _get_program()
    in_maps = _make_in_maps(inputs)
    res = run_bass_kernel_spmd(nc, in_maps, core_ids=list(range(NCORES)))
    _CACHE["last_results"] = res
    out = np.empty((BS, SEQ, D), np.float32)
    for core in range(NCORES):
        b, half = divmod(core, 2)
        out[b, half * S:(half + 1) * S, :] = res.results[core]["out"]
    return out


if __name__ == "__main__":
    rng = np.random.default_rng(0)
    fake = {
        "q": rng.standard_normal((BS, SEQ, D)).astype(np.float32),
        "k": rng.standard_normal((BS, SEQ, D)).astype(np.float32),
        "v": rng.standard_normal((BS, SEQ, D)).astype(np.float32),
        "wq_w": (rng.standard_normal((D * HEADS, D)) * 0.02).astype(np.float32),
        "wq_b": (rng.standard_normal((D * HEADS,)) * 0.02).astype(np.float32),
        "wk_w": (rng.standard_normal((D * HEADS, D)) * 0.02).astype(np.float32),
        "wk_b": (rng.standard_normal((D * HEADS,)) * 0.02).astype(np.float32),
        "wv_w": (rng.standard_normal((D * HEADS, D)) * 0.02).astype(np.float32),
        "wv_b": (rng.standard_normal((D * HEADS,)) * 0.02).astype(np.float32),
        "out_w": (rng.standard_normal((D, D * HEADS)) * 0.02).astype(np.float32),
        "out_b": (rng.standard_normal((D,)) * 0.02).astype(np.float32),
    }
    o = kernel(**fake)
    print("kernel ran, out shape", o.shape, "std", o.std())
